# revision 21
# baseline (speedup 1.0000x reference)
"""GQA attention layer (B=4, S=2048, D=2048, 16 heads / 4 KV heads, RoPE,
causal) on 8 trn2 NeuronCores.

Sharding: TP=4 over KV-head groups x DP=2 over batch. Each core handles 2
batches and one KV group (4 q heads + 1 kv head), computes a partial
(head-group) contribution to out = attn @ wo; host sums the 4 partials per
batch group.

Device layout choices:
  - host pre-casts to bf16, pre-transposes x -> xT [D, T], and permutes
    wq/wk columns per head to "evens then odds" so RoPE becomes rotate-half.
  - q,k are produced transposed ([dh, tok]) straight from the projection
    matmuls; v is produced natural ([tok, dh]).
  - RoPE in transposed layout: rot = q * C2 + swap_halves(q) * S2 with
    C2 = [cos;cos], S2 = [-sin;+sin]; the half swap is 2 SBUF->SBUF DMAs.
  - attention: scoresT = kT_tile.T @ qT (k on partitions), full 512-wide
    score tiles paired two-per-PSUM-group so one ACT exp covers [128,1024]
    (halves the 352-cycle ACT fixed cost). Causal masking is a 0/1
    multiplicative mask applied to e on the Vector engine (keeps PE free).
    Softmax denominator: e tiles pair+quad-added on DVE (bf16), one
    accumulated ones-matmul per quad; 1/l via ACT ln -> exp(-x).
  - wo PSUM->SBUF copies on DVE; wo PSUM tiles share a 2-buf pool with the
    l tiles so wo groups double-buffer (8 PSUM banks exactly).
"""

import math
from contextlib import ExitStack

import ml_dtypes
import numpy as np

import concourse.bass as bass
import concourse.mybir as mybir
import concourse.tile as tile
from concourse import bacc
from concourse.bass_utils import run_bass_kernel_spmd

BF16 = mybir.dt.bfloat16
F32 = mybir.dt.float32

# Full-problem constants (hardcoded per harness contract)
B, S, D = 4, 2048, 2048
NH, NKV, DH = 16, 4, 128
TP, DP = 4, 2
BL = B // DP          # batches per core
T = BL * S            # tokens per core
HL = NH // TP         # q heads per core
QC = HL * DH          # q cols per core
NT128 = S // 128      # 128-token tiles per batch (16)
NSL = S // 512        # 512-token slices per batch (4)
KD = D // 128         # contraction tiles for the projections (16)


def _patch_act_tables():
    """Make natural_log_exp_and_others the only set claiming Exp/Ln/Copy/
    Identity (everything this kernel runs on ACT) so the act-table-load pass
    emits exactly one load instead of switching sets between the projection
    phase (copies) and the attention phase (exp)."""
    if getattr(bacc, "_act_tables_patched", False):
        return
    orig = bacc.get_activation_tables

    def patched(arch):
        tabs = orig(arch)
        both = tabs.get("natural_log_exp_and_others")
        if both is None:
            return tabs
        claimed = [
            mybir.ActivationFunctionType.Exp,
            mybir.ActivationFunctionType.Ln,
            mybir.ActivationFunctionType.Copy,
            mybir.ActivationFunctionType.Identity,
        ]
        for name, s in tabs.items():
            if name != "natural_log_exp_and_others":
                for fn in claimed:
                    s.discard(fn)
        return tabs

    bacc.get_activation_tables = patched
    bacc._act_tables_patched = True


def build_nc(sc_bufs=2, oT_bufs=2, lw_bufs=2, exp_bufs=4, xt_bufs=2,
             q_bufs=2, k_bufs=2, v_bufs=2, psb_bufs=2, asb_bufs=3,
             ps_bufs=3):
    _patch_act_tables()
    nc = bacc.Bacc("TRN2", target_bir_lowering=False, debug=False)

    xt = nc.dram_tensor("xt", [D, T], BF16, kind="ExternalInput").ap()
    wq = nc.dram_tensor("wq", [D, QC], BF16, kind="ExternalInput").ap()
    wk = nc.dram_tensor("wk", [D, DH], BF16, kind="ExternalInput").ap()
    wv = nc.dram_tensor("wv", [D, DH], BF16, kind="ExternalInput").ap()
    wo = nc.dram_tensor("wo", [QC, D], BF16, kind="ExternalInput").ap()
    cos2 = nc.dram_tensor("cos2", [DH, S], BF16, kind="ExternalInput").ap()
    sin2 = nc.dram_tensor("sin2", [DH, S], BF16, kind="ExternalInput").ap()
    m01 = nc.dram_tensor("m01", [4, 128, 512], BF16, kind="ExternalInput").ap()
    out = nc.dram_tensor("out", [T, D], F32, kind="ExternalOutput").ap()

    scale = 1.0 / math.sqrt(DH)
    Exp = mybir.ActivationFunctionType.Exp
    Ln = mybir.ActivationFunctionType.Ln

    with tile.TileContext(nc) as tc, ExitStack() as ctx:
        persist = ctx.enter_context(tc.tile_pool(name="persist", bufs=1))

        # --- resident weights / tables. Two HWDGE rings: xt slices (and
        # rope swaps / out) go on the sync(SP) ring; everything the first
        # si iteration doesn't immediately need rides the ACT ring so
        # xt slice 0 isn't queued behind 6MB of weights. ---
        wk_sb = persist.tile([128, KD, DH], BF16, tag="wk")
        wk_r = wk.rearrange("(o p) c -> p o c", p=128)
        cos_sb = persist.tile([128, S], BF16, tag="cos")
        nc.scalar.dma_start(cos_sb[:], cos2)
        sin_sb = persist.tile([128, S], BF16, tag="sin")
        nc.scalar.dma_start(sin_sb[:], sin2)
        wv_sb = persist.tile([128, KD, DH], BF16, tag="wv")
        nc.scalar.dma_start(wv_sb[:], wv.rearrange("(o p) c -> p o c", p=128))
        wq_sb = persist.tile([128, KD, QC], BF16, tag="wq")
        wq_r = wq.rearrange("(o p) c -> p o c", p=128)
        for h in range(HL):
            # per-head slices so q(h=0) of si=0 doesn't wait for all of wq
            nc.scalar.dma_start(
                wq_sb[:, :, bass.ts(h, DH)], wq_r[:, :, bass.ts(h, DH)]
            )
        msk_sb = persist.tile([128, 4, 512], BF16, tag="m01")
        nc.scalar.dma_start(msk_sb[:], m01.rearrange("r p q -> p r q"))
        wo_sb = persist.tile([128, HL, D], BF16, tag="wo")
        nc.scalar.dma_start(wo_sb[:], wo.rearrange("(o p) f -> p o f", p=128))
        ones_sb = persist.tile([128, 128], BF16, tag="ones")
        nc.vector.memset(ones_sb[:], 1.0)

        # --- resident activations (one tile per (h,b) / per b so phase C's
        # first reads only depend on the exact producer, not all of phase B) ---
        qT_sb = {
            (h, b): persist.tile([128, S], BF16, tag=f"qT{h}{b}",
                                 name=f"qT_sb{h}{b}")
            for h in range(HL) for b in range(BL)
        }
        kT_sb = {
            b: persist.tile([128, S], BF16, tag=f"kT{b}", name=f"kT_sb{b}")
            for b in range(BL)
        }
        v_sb = {
            b: persist.tile([128, NT128, DH], BF16, tag=f"v{b}", name=f"v_sb{b}")
            for b in range(BL)
        }

        # ---------------- phase B: projections + RoPE ----------------
        with tc.tile_pool(name="proj_sb", bufs=psb_bufs) as psb, \
             tc.tile_pool(name="proj_ps", bufs=2, space="PSUM") as pps:

            def rope(dst, raw_ps, pos_sl):
                """dst[128,512] <- RoPE(raw_ps[128,512] psum), via bf16 sbuf."""
                raw = psb.tile([128, 512], BF16, tag="rraw")
                nc.scalar.copy(raw[:], raw_ps[:])
                swp = psb.tile([128, 512], BF16, tag="rswp")
                nc.sync.dma_start(swp[0:64, :], raw[64:128, :])
                nc.sync.dma_start(swp[64:128, :], raw[0:64, :])
                t1 = psb.tile([128, 512], BF16, tag="rt1")
                nc.vector.tensor_mul(t1[:], raw[:], cos_sb[:, pos_sl])
                t2 = psb.tile([128, 512], BF16, tag="rt2")
                nc.vector.tensor_mul(t2[:], swp[:], sin_sb[:, pos_sl])
                nc.vector.tensor_add(dst, t1[:], t2[:])

            for si in range(T // 512):
                b, sl = divmod(si, NSL)
                pos_sl = bass.ts(sl, 512)
                xt_sl = psb.tile([128, KD, 512], BF16, tag="xt", bufs=xt_bufs)
                xt_r = xt[:, bass.ts(si, 512)].rearrange("(o p) t -> p o t", p=128)
                if si == 0:
                    # per-o chunks interleaved with wk chunks: the k matmul
                    # accumulation starts once (wk0, xt0) land (~2µs)
                    # instead of after the full 2.5MB
                    for o in range(KD):
                        nc.sync.dma_start(wk_sb[:, o, :], wk_r[:, o, :])
                        nc.sync.dma_start(xt_sl[:, o, :], xt_r[:, o, :])
                else:
                    nc.sync.dma_start(xt_sl[:], xt_r)
                # k first: needs only wk (+cos/sin) resident
                k_ps = pps.tile([128, 512], F32, tag="k", bufs=k_bufs)
                for o in range(KD):
                    nc.tensor.matmul(
                        k_ps[:], wk_sb[:, o, :], xt_sl[:, o, :],
                        start=(o == 0), stop=(o == KD - 1),
                    )
                rope(kT_sb[b][:, pos_sl], k_ps, pos_sl)
                for jt in range(4):
                    v_ps = pps.tile([128, DH], F32, tag="v", bufs=v_bufs)
                    for o in range(KD):
                        nc.tensor.matmul(
                            v_ps[:], xt_sl[:, o, bass.ts(jt, 128)], wv_sb[:, o, :],
                            start=(o == 0), stop=(o == KD - 1),
                        )
                    nc.scalar.copy(v_sb[b][:, 4 * sl + jt, :], v_ps[:])
                for h in range(HL):
                    q_ps = pps.tile([128, 512], F32, tag="q", bufs=q_bufs)
                    for o in range(KD):
                        nc.tensor.matmul(
                            q_ps[:], wq_sb[:, o, bass.ts(h, DH)], xt_sl[:, o, :],
                            start=(o == 0), stop=(o == KD - 1),
                        )
                    rope(qT_sb[h, b][:, pos_sl], q_ps, pos_sl)

        # ---------------- phase C: attention + wo ----------------
        with tc.tile_pool(name="att_sb", bufs=asb_bufs) as asb, \
             tc.tile_pool(name="att_sb2", bufs=2) as asb2, \
             tc.tile_pool(name="att_ps", bufs=2, space="PSUM") as aps:

            def wo_group(b, qs, aoT, nt, od, tag="wo", copy_eng="v"):
                """one wo matmul group + out-DMA for 128 tokens x 512 dims."""
                w_ps = aps.tile([128, 512], F32, tag=tag, name="w_ps", bufs=1)
                for c in range(HL):
                    nc.tensor.matmul(
                        w_ps[:], aoT[:, c, bass.ts(nt, 128)],
                        wo_sb[:, c, bass.ts(od, 512)],
                        start=(c == 0), stop=(c == HL - 1),
                    )
                o_sb = asb.tile([128, 512], F32, tag="out")
                if copy_eng == "v":
                    nc.vector.tensor_copy(o_sb[:], w_ps[:])
                else:
                    nc.scalar.copy(o_sb[:], w_ps[:])
                nc.sync.dma_start(
                    out[
                        bass.ds(b * S + qs * 512 + nt * 128, 128),
                        bass.ts(od, 512),
                    ],
                    o_sb[:],
                )

            # wo groups of block i are spread evenly between the attention
            # pairs of block i+1 so PE always has independent matmul work
            # while the softmax pipeline (exp on ACT, adds/recip on DVE)
            # catches up; copies alternate DVE/ACT to balance those engines
            pending_wo = []
            for b in range(BL):
                for qs in range(NSL):
                    q_sl = bass.ts(qs, 512)
                    nk = 4 * qs + 4
                    npair = nk // 2
                    nquad = nk // 4
                    total_pairs = HL * npair
                    # emit pending_wo[i] when the block-wide pair counter
                    # reaches emit_at[i] (even distribution across the block)
                    emit_at = [
                        (i * total_pairs) // max(len(pending_wo), 1)
                        for i in range(len(pending_wo))
                    ]
                    pair_ctr = 0
                    aoT = asb2.tile([128, HL, 512], BF16, tag="aoT")
                    for h in range(HL):
                        oT_ps = aps.tile([128, 512], F32, tag="oT", bufs=oT_bufs)
                        l_ps = aps.tile(
                            [128, 512], F32, tag="l", name="l_ps", bufs=1
                        )
                        pend = None  # pair half-sum awaiting its quad partner
                        for p in range(npair):
                            j0 = 2 * p
                            s_ps = aps.tile([128, 1024], F32, tag="sc", bufs=sc_bufs)
                            e_sb = asb.tile(
                                [128, 1024], BF16, tag="exp", bufs=exp_bufs
                            )
                            for t in range(2):
                                j = j0 + t
                                nc.tensor.matmul(
                                    s_ps[:, bass.ts(t, 512)],
                                    kT_sb[b][:, bass.ts(j, 128)],
                                    qT_sb[h, b][:, q_sl],
                                    start=True, stop=True, skip_group_check=True,
                                )
                            # one exp for both k-tiles: [128,1024]
                            nc.scalar.activation(e_sb[:], s_ps[:], Exp, scale=scale)
                            for t in range(2):
                                j = j0 + t
                                r = j - 4 * qs
                                if r >= 0:
                                    # zero the causally-invalid region
                                    # (cols [0, 128(r+1)) of this half)
                                    w = 128 * (r + 1)
                                    sl_ = bass.ds(512 * t, w)
                                    nc.vector.tensor_mul(
                                        e_sb[:, sl_], e_sb[:, sl_],
                                        msk_sb[:, r, 0:w],
                                    )
                            for t in range(2):
                                j = j0 + t
                                r = j - 4 * qs
                                qlo = 128 * r if r > 0 else 0
                                nc.tensor.matmul(
                                    oT_ps[:, qlo:], v_sb[b][:, j, :],
                                    e_sb[:, bass.ds(512 * t + qlo, 512 - qlo)],
                                    start=(j == 0), stop=(j == nk - 1),
                                    skip_group_check=True,
                                )
                            # softmax denominator: pair/quad tree on DVE,
                            # one accumulated ones-matmul per quad
                            t1 = asb.tile([128, 512], BF16, tag="ps1", bufs=ps_bufs)
                            nc.vector.tensor_add(
                                t1[:], e_sb[:, 0:512], e_sb[:, 512:1024]
                            )
                            if pend is None:
                                pend = t1
                            else:
                                t3 = asb.tile([128, 512], BF16, tag="ps2", bufs=2)
                                nc.vector.tensor_add(t3[:], pend[:], t1[:])
                                pend = None
                                qd = p // 2
                                nc.tensor.matmul(
                                    l_ps[:], ones_sb[:], t3[:],
                                    start=(qd == 0), stop=(qd == nquad - 1),
                                    skip_group_check=True,
                                )
                            while pending_wo and emit_at and pair_ctr >= emit_at[0]:
                                emit_at.pop(0)
                                pending_wo.pop(0)()
                            pair_ctr += 1
                        rc_sb = asb.tile([128, 512], F32, tag="rc")
                        nc.vector.reciprocal_approx_fast(rc_sb[:], l_ps[:])
                        nc.vector.tensor_mul(aoT[:, h, :], oT_ps[:], rc_sb[:])
                    while pending_wo:
                        pending_wo.pop(0)()
                    pending_wo = [
                        (lambda b=b, qs=qs, aoT=aoT, nt=nt, od=od, i=4 * nt + od:
                         wo_group(b, qs, aoT, nt, od,
                                  copy_eng="v" if i % 2 else "s"))
                        for nt in range(4) for od in range(4)
                    ]
            # final block's wo: alternate the two single-buf PSUM tags so
            # group i+1's matmuls overlap group i's copy (no attention work
            # is left to fill the gap here)
            for i, (nt, od) in enumerate(
                (nt, od) for nt in range(4) for od in range(4)
            ):
                wo_group(BL - 1, NSL - 1, aoT, nt, od,
                         tag="wo" if i % 2 else "l",
                         copy_eng="v" if i % 2 else "s")
    nc.finalize()
    return nc


_NC_CACHE = {}


def _get_nc():
    if "nc" not in _NC_CACHE:
        _NC_CACHE["nc"] = build_nc()
    return _NC_CACHE["nc"]


def kernel(x, freqs_cos, freqs_sin, wq, wk, wv, wo):
    x = np.asarray(x)
    freqs_cos = np.asarray(freqs_cos)
    freqs_sin = np.asarray(freqs_sin)
    wq = np.asarray(wq)
    wk = np.asarray(wk)
    wv = np.asarray(wv)
    wo = np.asarray(wo)
    bf = ml_dtypes.bfloat16
    perm = np.concatenate([np.arange(0, DH, 2), np.arange(1, DH, 2)])

    wq_p = np.ascontiguousarray(
        wq.reshape(D, NH, DH)[:, :, perm].reshape(D, NH * DH)
    ).astype(bf)
    wk_p = np.ascontiguousarray(
        wk.reshape(D, NKV, DH)[:, :, perm].reshape(D, NKV * DH)
    ).astype(bf)
    wv_b = wv.astype(bf)
    wo_b = wo.astype(bf)

    cosT = freqs_cos.T  # [64, S]
    sinT = freqs_sin.T
    c2 = np.ascontiguousarray(np.concatenate([cosT, cosT], axis=0)).astype(bf)
    s2 = np.ascontiguousarray(np.concatenate([-sinT, sinT], axis=0)).astype(bf)

    # multiplicative mask[r][k, q] = 1 if causally valid (q - k - 128 r >= 0)
    # else 0; applied to e = exp(scores) on DVE
    kk = np.arange(128)[:, None]
    qq = np.arange(512)[None, :]
    masks = np.stack(
        [np.where(qq - kk - 128 * r >= 0, 1.0, 0.0) for r in range(4)]
    ).astype(bf)

    in_maps = []
    for core in range(8):
        dp, tp = divmod(core, TP)
        xs = x[dp * BL : (dp + 1) * BL].reshape(T, D)
        xt = np.ascontiguousarray(xs.T).astype(bf)
        in_maps.append(
            {
                "xt": xt,
                "wq": np.ascontiguousarray(wq_p[:, tp * QC : (tp + 1) * QC]),
                "wk": np.ascontiguousarray(wk_p[:, tp * DH : (tp + 1) * DH]),
                "wv": np.ascontiguousarray(wv_b[:, tp * DH : (tp + 1) * DH]),
                "wo": np.ascontiguousarray(wo_b[tp * QC : (tp + 1) * QC, :]),
                "cos2": c2,
                "sin2": s2,
                "m01": masks,
            }
        )

    nc = _get_nc()
    res = run_bass_kernel_spmd(nc, in_maps, core_ids=list(range(8)))
    _NC_CACHE["last_results"] = res

    full = np.zeros((B, S, D), dtype=np.float32)
    for core in range(8):
        dp = core // TP
        full[dp * BL : (dp + 1) * BL] += (
            res.results[core]["out"].astype(np.float32).reshape(BL, S, D)
        )
    return full


# revision 25
# speedup vs baseline: 1.0106x; 1.0106x over previous
"""GQA attention layer (B=4, S=2048, D=2048, 16 heads / 4 KV heads, RoPE,
causal) on 8 trn2 NeuronCores.

Sharding: TP=4 over KV-head groups x DP=2 over batch. Each core handles 2
batches and one KV group (4 q heads + 1 kv head), computes a partial
(head-group) contribution to out = attn @ wo; host sums the 4 partials per
batch group.

Device layout choices:
  - host pre-casts to bf16, pre-transposes x -> xT [D, T], and permutes
    wq/wk columns per head to "evens then odds" so RoPE becomes rotate-half.
  - q,k are produced transposed ([dh, tok]) straight from the projection
    matmuls; v is produced natural ([tok, dh]).
  - RoPE in transposed layout: rot = q * C2 + swap_halves(q) * S2 with
    C2 = [cos;cos], S2 = [-sin;+sin]; the half swap is 2 SBUF->SBUF DMAs.
  - attention: scoresT = kT_tile.T @ qT (k on partitions), full 512-wide
    score tiles paired two-per-PSUM-group so one ACT exp covers [128,1024]
    (halves the 352-cycle ACT fixed cost). Causal masking is a 0/1
    multiplicative mask applied to e on the Vector engine (keeps PE free).
    Softmax denominator: e tiles pair+quad-added on DVE (bf16), one
    accumulated ones-matmul per quad; 1/l via ACT ln -> exp(-x).
  - wo PSUM->SBUF copies on DVE; wo PSUM tiles share a 2-buf pool with the
    l tiles so wo groups double-buffer (8 PSUM banks exactly).
"""

import math
from contextlib import ExitStack

import ml_dtypes
import numpy as np

import concourse.bass as bass
import concourse.mybir as mybir
import concourse.tile as tile
from concourse import bacc
from concourse.bass_utils import run_bass_kernel_spmd

BF16 = mybir.dt.bfloat16
F32 = mybir.dt.float32

# Full-problem constants (hardcoded per harness contract)
B, S, D = 4, 2048, 2048
NH, NKV, DH = 16, 4, 128
TP, DP = 4, 2
BL = B // DP          # batches per core
T = BL * S            # tokens per core
HL = NH // TP         # q heads per core
QC = HL * DH          # q cols per core
NT128 = S // 128      # 128-token tiles per batch (16)
NSL = S // 512        # 512-token slices per batch (4)
KD = D // 128         # contraction tiles for the projections (16)


def _patch_act_tables():
    """Make natural_log_exp_and_others the only set claiming Exp/Ln/Copy/
    Identity (everything this kernel runs on ACT) so the act-table-load pass
    emits exactly one load instead of switching sets between the projection
    phase (copies) and the attention phase (exp)."""
    if getattr(bacc, "_act_tables_patched", False):
        return
    orig = bacc.get_activation_tables

    def patched(arch):
        tabs = orig(arch)
        both = tabs.get("natural_log_exp_and_others")
        if both is None:
            return tabs
        claimed = [
            mybir.ActivationFunctionType.Exp,
            mybir.ActivationFunctionType.Ln,
            mybir.ActivationFunctionType.Copy,
            mybir.ActivationFunctionType.Identity,
        ]
        for name, s in tabs.items():
            if name != "natural_log_exp_and_others":
                for fn in claimed:
                    s.discard(fn)
        return tabs

    bacc.get_activation_tables = patched
    bacc._act_tables_patched = True


def build_nc(sc_bufs=2, oT_bufs=2, lw_bufs=2, exp_bufs=4, xt_bufs=2,
             q_bufs=2, k_bufs=2, v_bufs=2, psb_bufs=2, asb_bufs=3,
             ps_bufs=3):
    _patch_act_tables()
    nc = bacc.Bacc("TRN2", target_bir_lowering=False, debug=False)

    xt = nc.dram_tensor("xt", [D, T], BF16, kind="ExternalInput").ap()
    wq = nc.dram_tensor("wq", [D, QC], BF16, kind="ExternalInput").ap()
    wk = nc.dram_tensor("wk", [D, DH], BF16, kind="ExternalInput").ap()
    wv = nc.dram_tensor("wv", [D, DH], BF16, kind="ExternalInput").ap()
    wo = nc.dram_tensor("wo", [QC, D], BF16, kind="ExternalInput").ap()
    cos2 = nc.dram_tensor("cos2", [DH, S], BF16, kind="ExternalInput").ap()
    sin2 = nc.dram_tensor("sin2", [DH, S], BF16, kind="ExternalInput").ap()
    m01 = nc.dram_tensor("m01", [4, 128, 512], BF16, kind="ExternalInput").ap()
    out = nc.dram_tensor("out", [T, D], F32, kind="ExternalOutput").ap()

    scale = 1.0 / math.sqrt(DH)
    Exp = mybir.ActivationFunctionType.Exp
    Ln = mybir.ActivationFunctionType.Ln

    with tile.TileContext(nc) as tc, ExitStack() as ctx:
        persist = ctx.enter_context(tc.tile_pool(name="persist", bufs=1))

        # --- resident weights / tables. Two HWDGE rings: xt slices (and
        # rope swaps / out) go on the sync(SP) ring; everything the first
        # si iteration doesn't immediately need rides the ACT ring so
        # xt slice 0 isn't queued behind 6MB of weights. ---
        wk_sb = persist.tile([128, KD, DH], BF16, tag="wk")
        nc.sync.dma_start(wk_sb[:], wk.rearrange("(o p) c -> p o c", p=128))
        cos_sb = persist.tile([128, S], BF16, tag="cos")
        nc.scalar.dma_start(cos_sb[:], cos2)
        sin_sb = persist.tile([128, S], BF16, tag="sin")
        nc.scalar.dma_start(sin_sb[:], sin2)
        wv_sb = persist.tile([128, KD, DH], BF16, tag="wv")
        nc.scalar.dma_start(wv_sb[:], wv.rearrange("(o p) c -> p o c", p=128))
        wq_sb = persist.tile([128, KD, QC], BF16, tag="wq")
        wq_r = wq.rearrange("(o p) c -> p o c", p=128)
        for h in range(HL):
            # per-head slices so q(h=0) of si=0 doesn't wait for all of wq
            nc.scalar.dma_start(
                wq_sb[:, :, bass.ts(h, DH)], wq_r[:, :, bass.ts(h, DH)]
            )
        msk_sb = persist.tile([128, 4, 512], BF16, tag="m01")
        nc.scalar.dma_start(msk_sb[:], m01.rearrange("r p q -> p r q"))
        wo_sb = persist.tile([128, HL, D], BF16, tag="wo")
        nc.scalar.dma_start(wo_sb[:], wo.rearrange("(o p) f -> p o f", p=128))
        ones_sb = persist.tile([128, 128], BF16, tag="ones")
        nc.vector.memset(ones_sb[:], 1.0)

        # --- resident activations (one tile per (h,b) / per b so phase C's
        # first reads only depend on the exact producer, not all of phase B) ---
        qT_sb = {
            (h, b): persist.tile([128, S], BF16, tag=f"qT{h}{b}",
                                 name=f"qT_sb{h}{b}")
            for h in range(HL) for b in range(BL)
        }
        kT_sb = {
            b: persist.tile([128, S], BF16, tag=f"kT{b}", name=f"kT_sb{b}")
            for b in range(BL)
        }
        v_sb = {
            b: persist.tile([128, NT128, DH], BF16, tag=f"v{b}", name=f"v_sb{b}")
            for b in range(BL)
        }

        # ---------------- phase B: projections + RoPE ----------------
        with tc.tile_pool(name="proj_sb", bufs=psb_bufs) as psb, \
             tc.tile_pool(name="proj_ps", bufs=2, space="PSUM") as pps:

            def rope(dst, raw_ps, pos_sl):
                """dst[128,512] <- RoPE(raw_ps[128,512] psum), via bf16 sbuf."""
                raw = psb.tile([128, 512], BF16, tag="rraw")
                nc.scalar.copy(raw[:], raw_ps[:])
                swp = psb.tile([128, 512], BF16, tag="rswp")
                nc.sync.dma_start(swp[0:64, :], raw[64:128, :])
                nc.sync.dma_start(swp[64:128, :], raw[0:64, :])
                t1 = psb.tile([128, 512], BF16, tag="rt1")
                nc.vector.tensor_mul(t1[:], raw[:], cos_sb[:, pos_sl])
                t2 = psb.tile([128, 512], BF16, tag="rt2")
                nc.vector.tensor_mul(t2[:], swp[:], sin_sb[:, pos_sl])
                nc.vector.tensor_add(dst, t1[:], t2[:])

            for si in range(T // 512):
                b, sl = divmod(si, NSL)
                pos_sl = bass.ts(sl, 512)
                xt_sl = psb.tile([128, KD, 512], BF16, tag="xt", bufs=xt_bufs)
                xt_r = xt[:, bass.ts(si, 512)].rearrange("(o p) t -> p o t", p=128)
                if si == 0:
                    # quarter chunks: the k matmul accumulation starts once
                    # the first 0.5MB lands instead of after the full 2MB
                    for o4 in range(4):
                        nc.sync.dma_start(
                            xt_sl[:, bass.ts(o4, 4), :], xt_r[:, bass.ts(o4, 4), :]
                        )
                else:
                    nc.sync.dma_start(xt_sl[:], xt_r)
                # k first: needs only wk (+cos/sin) resident
                k_ps = pps.tile([128, 512], F32, tag="k", bufs=k_bufs)
                for o in range(KD):
                    nc.tensor.matmul(
                        k_ps[:], wk_sb[:, o, :], xt_sl[:, o, :],
                        start=(o == 0), stop=(o == KD - 1),
                    )
                rope(kT_sb[b][:, pos_sl], k_ps, pos_sl)
                for jt in range(4):
                    v_ps = pps.tile([128, DH], F32, tag="v", bufs=v_bufs)
                    for o in range(KD):
                        nc.tensor.matmul(
                            v_ps[:], xt_sl[:, o, bass.ts(jt, 128)], wv_sb[:, o, :],
                            start=(o == 0), stop=(o == KD - 1),
                        )
                    nc.scalar.copy(v_sb[b][:, 4 * sl + jt, :], v_ps[:])
                for h in range(HL):
                    q_ps = pps.tile([128, 512], F32, tag="q", bufs=q_bufs)
                    for o in range(KD):
                        nc.tensor.matmul(
                            q_ps[:], wq_sb[:, o, bass.ts(h, DH)], xt_sl[:, o, :],
                            start=(o == 0), stop=(o == KD - 1),
                        )
                    rope(qT_sb[h, b][:, pos_sl], q_ps, pos_sl)

        # ---------------- phase C: attention + wo ----------------
        with tc.tile_pool(name="att_sb", bufs=asb_bufs) as asb, \
             tc.tile_pool(name="att_sb2", bufs=2) as asb2, \
             tc.tile_pool(name="att_ps", bufs=2, space="PSUM") as aps:

            def wo_group(b, qs, aoT, nt, od, tag="wo", copy_eng="v"):
                """one wo matmul group + out-DMA for 128 tokens x 512 dims."""
                w_ps = aps.tile([128, 512], F32, tag=tag, name="w_ps", bufs=1)
                for c in range(HL):
                    nc.tensor.matmul(
                        w_ps[:], aoT[:, c, bass.ts(nt, 128)],
                        wo_sb[:, c, bass.ts(od, 512)],
                        start=(c == 0), stop=(c == HL - 1),
                    )
                o_sb = asb.tile([128, 512], F32, tag="out")
                if copy_eng == "v":
                    nc.vector.tensor_copy(o_sb[:], w_ps[:])
                else:
                    nc.scalar.copy(o_sb[:], w_ps[:])
                nc.sync.dma_start(
                    out[
                        bass.ds(b * S + qs * 512 + nt * 128, 128),
                        bass.ts(od, 512),
                    ],
                    o_sb[:],
                )

            # wo groups of block i are spread evenly between the attention
            # pairs of block i+1 so PE always has independent matmul work
            # while the softmax pipeline (exp on ACT, adds/recip on DVE)
            # catches up; copies alternate DVE/ACT to balance those engines
            pending_wo = []
            for b in range(BL):
                for qs in range(NSL):
                    q_sl = bass.ts(qs, 512)
                    nk = 4 * qs + 4
                    npair = nk // 2
                    nquad = nk // 4
                    total_pairs = HL * npair
                    # emit pending_wo[i] when the block-wide pair counter
                    # reaches emit_at[i] (even distribution across the block)
                    emit_at = [
                        (i * total_pairs) // max(len(pending_wo), 1)
                        for i in range(len(pending_wo))
                    ]
                    pair_ctr = 0
                    aoT = asb2.tile([128, HL, 512], BF16, tag="aoT")
                    for h in range(HL):
                        oT_ps = aps.tile([128, 512], F32, tag="oT", bufs=oT_bufs)
                        l_ps = aps.tile(
                            [128, 512], F32, tag="l", name="l_ps", bufs=1
                        )
                        pend = None  # pair half-sum awaiting its quad partner
                        for p in range(npair):
                            j0 = 2 * p
                            s_ps = aps.tile([128, 1024], F32, tag="sc", bufs=sc_bufs)
                            e_sb = asb.tile(
                                [128, 1024], BF16, tag="exp", bufs=exp_bufs
                            )
                            for t in range(2):
                                j = j0 + t
                                nc.tensor.matmul(
                                    s_ps[:, bass.ts(t, 512)],
                                    kT_sb[b][:, bass.ts(j, 128)],
                                    qT_sb[h, b][:, q_sl],
                                    start=True, stop=True, skip_group_check=True,
                                )
                            # one exp for both k-tiles: [128,1024]
                            nc.scalar.activation(e_sb[:], s_ps[:], Exp, scale=scale)
                            for t in range(2):
                                j = j0 + t
                                r = j - 4 * qs
                                if r >= 0:
                                    # zero the causally-invalid region
                                    # (cols [0, 128(r+1)) of this half)
                                    w = 128 * (r + 1)
                                    sl_ = bass.ds(512 * t, w)
                                    nc.vector.tensor_mul(
                                        e_sb[:, sl_], e_sb[:, sl_],
                                        msk_sb[:, r, 0:w],
                                    )
                            for t in range(2):
                                j = j0 + t
                                r = j - 4 * qs
                                qlo = 128 * r if r > 0 else 0
                                nc.tensor.matmul(
                                    oT_ps[:, qlo:], v_sb[b][:, j, :],
                                    e_sb[:, bass.ds(512 * t + qlo, 512 - qlo)],
                                    start=(j == 0), stop=(j == nk - 1),
                                    skip_group_check=True,
                                )
                            # softmax denominator: pair/quad tree on DVE,
                            # one accumulated ones-matmul per quad
                            t1 = asb.tile([128, 512], BF16, tag="ps1", bufs=ps_bufs)
                            nc.vector.tensor_add(
                                t1[:], e_sb[:, 0:512], e_sb[:, 512:1024]
                            )
                            if pend is None:
                                pend = t1
                            else:
                                t3 = asb.tile([128, 512], BF16, tag="ps2", bufs=2)
                                nc.vector.tensor_add(t3[:], pend[:], t1[:])
                                pend = None
                                qd = p // 2
                                nc.tensor.matmul(
                                    l_ps[:], ones_sb[:], t3[:],
                                    start=(qd == 0), stop=(qd == nquad - 1),
                                    skip_group_check=True,
                                )
                            while pending_wo and emit_at and pair_ctr >= emit_at[0]:
                                emit_at.pop(0)
                                pending_wo.pop(0)()
                            pair_ctr += 1
                        rc_sb = asb.tile([128, 512], F32, tag="rc")
                        nc.vector.reciprocal_approx_fast(rc_sb[:], l_ps[:])
                        nc.vector.tensor_mul(aoT[:, h, :], oT_ps[:], rc_sb[:])
                    while pending_wo:
                        pending_wo.pop(0)()
                    pending_wo = [
                        (lambda b=b, qs=qs, aoT=aoT, nt=nt, od=od, i=4 * nt + od:
                         wo_group(b, qs, aoT, nt, od,
                                  copy_eng="v" if i % 2 else "s"))
                        for nt in range(4) for od in range(4)
                    ]
            # keep-warm matmuls: PE idles ~4-5µs here waiting for the last
            # head's softmax tail before the final wo flush; that idle
            # re-throttles the HAM clock gate to 1.2GHz for the whole flush.
            # These dummies (results never read) fill the idle window.
            warm_ps = aps.tile(
                [128, 512], F32, tag="sc", name="warm_ps", bufs=sc_bufs
            )
            for w in range(20):
                nc.tensor.matmul(
                    warm_ps[:], ones_sb[:], kT_sb[0][:, 0:512],
                    start=True, stop=True, skip_group_check=True,
                )
            # final block's wo: alternate the two single-buf PSUM tags so
            # group i+1's matmuls overlap group i's copy (no attention work
            # is left to fill the gap here)
            for i, (nt, od) in enumerate(
                (nt, od) for nt in range(4) for od in range(4)
            ):
                wo_group(BL - 1, NSL - 1, aoT, nt, od,
                         tag="wo" if i % 2 else "l",
                         copy_eng="v" if i % 2 else "s")
    nc.finalize()
    return nc


_NC_CACHE = {}


def _get_nc():
    if "nc" not in _NC_CACHE:
        _NC_CACHE["nc"] = build_nc()
    return _NC_CACHE["nc"]


def kernel(x, freqs_cos, freqs_sin, wq, wk, wv, wo):
    x = np.asarray(x)
    freqs_cos = np.asarray(freqs_cos)
    freqs_sin = np.asarray(freqs_sin)
    wq = np.asarray(wq)
    wk = np.asarray(wk)
    wv = np.asarray(wv)
    wo = np.asarray(wo)
    bf = ml_dtypes.bfloat16
    perm = np.concatenate([np.arange(0, DH, 2), np.arange(1, DH, 2)])

    wq_p = np.ascontiguousarray(
        wq.reshape(D, NH, DH)[:, :, perm].reshape(D, NH * DH)
    ).astype(bf)
    wk_p = np.ascontiguousarray(
        wk.reshape(D, NKV, DH)[:, :, perm].reshape(D, NKV * DH)
    ).astype(bf)
    wv_b = wv.astype(bf)
    wo_b = wo.astype(bf)

    cosT = freqs_cos.T  # [64, S]
    sinT = freqs_sin.T
    c2 = np.ascontiguousarray(np.concatenate([cosT, cosT], axis=0)).astype(bf)
    s2 = np.ascontiguousarray(np.concatenate([-sinT, sinT], axis=0)).astype(bf)

    # multiplicative mask[r][k, q] = 1 if causally valid (q - k - 128 r >= 0)
    # else 0; applied to e = exp(scores) on DVE
    kk = np.arange(128)[:, None]
    qq = np.arange(512)[None, :]
    masks = np.stack(
        [np.where(qq - kk - 128 * r >= 0, 1.0, 0.0) for r in range(4)]
    ).astype(bf)

    in_maps = []
    for core in range(8):
        dp, tp = divmod(core, TP)
        xs = x[dp * BL : (dp + 1) * BL].reshape(T, D)
        xt = np.ascontiguousarray(xs.T).astype(bf)
        in_maps.append(
            {
                "xt": xt,
                "wq": np.ascontiguousarray(wq_p[:, tp * QC : (tp + 1) * QC]),
                "wk": np.ascontiguousarray(wk_p[:, tp * DH : (tp + 1) * DH]),
                "wv": np.ascontiguousarray(wv_b[:, tp * DH : (tp + 1) * DH]),
                "wo": np.ascontiguousarray(wo_b[tp * QC : (tp + 1) * QC, :]),
                "cos2": c2,
                "sin2": s2,
                "m01": masks,
            }
        )

    nc = _get_nc()
    res = run_bass_kernel_spmd(nc, in_maps, core_ids=list(range(8)))
    _NC_CACHE["last_results"] = res

    full = np.zeros((B, S, D), dtype=np.float32)
    for core in range(8):
        dp = core // TP
        full[dp * BL : (dp + 1) * BL] += (
            res.results[core]["out"].astype(np.float32).reshape(BL, S, D)
        )
    return full


# revision 27
# speedup vs baseline: 1.0156x; 1.0050x over previous
"""GQA attention layer (B=4, S=2048, D=2048, 16 heads / 4 KV heads, RoPE,
causal) on 8 trn2 NeuronCores.

Sharding: TP=4 over KV-head groups x DP=2 over batch. Each core handles 2
batches and one KV group (4 q heads + 1 kv head), computes a partial
(head-group) contribution to out = attn @ wo; host sums the 4 partials per
batch group.

Device layout choices:
  - host pre-casts to bf16, pre-transposes x -> xT [D, T], and permutes
    wq/wk columns per head to "evens then odds" so RoPE becomes rotate-half.
  - q,k are produced transposed ([dh, tok]) straight from the projection
    matmuls; v is produced natural ([tok, dh]).
  - RoPE in transposed layout: rot = q * C2 + swap_halves(q) * S2 with
    C2 = [cos;cos], S2 = [-sin;+sin]; the half swap is 2 SBUF->SBUF DMAs.
  - attention: scoresT = kT_tile.T @ qT (k on partitions), full 512-wide
    score tiles paired two-per-PSUM-group so one ACT exp covers [128,1024]
    (halves the 352-cycle ACT fixed cost). Causal masking is a 0/1
    multiplicative mask applied to e on the Vector engine (keeps PE free).
    Softmax denominator: e tiles pair+quad-added on DVE (bf16), one
    accumulated ones-matmul per quad; 1/l via ACT ln -> exp(-x).
  - wo PSUM->SBUF copies on DVE; wo PSUM tiles share a 2-buf pool with the
    l tiles so wo groups double-buffer (8 PSUM banks exactly).
"""

import math
from contextlib import ExitStack

import ml_dtypes
import numpy as np

import concourse.bass as bass
import concourse.mybir as mybir
import concourse.tile as tile
from concourse import bacc
from concourse.bass_utils import run_bass_kernel_spmd

BF16 = mybir.dt.bfloat16
F32 = mybir.dt.float32

# Full-problem constants (hardcoded per harness contract)
B, S, D = 4, 2048, 2048
NH, NKV, DH = 16, 4, 128
TP, DP = 4, 2
BL = B // DP          # batches per core
T = BL * S            # tokens per core
HL = NH // TP         # q heads per core
QC = HL * DH          # q cols per core
NT128 = S // 128      # 128-token tiles per batch (16)
NSL = S // 512        # 512-token slices per batch (4)
KD = D // 128         # contraction tiles for the projections (16)


def _patch_act_tables():
    """Make natural_log_exp_and_others the only set claiming Exp/Ln/Copy/
    Identity (everything this kernel runs on ACT) so the act-table-load pass
    emits exactly one load instead of switching sets between the projection
    phase (copies) and the attention phase (exp)."""
    if getattr(bacc, "_act_tables_patched", False):
        return
    orig = bacc.get_activation_tables

    def patched(arch):
        tabs = orig(arch)
        both = tabs.get("natural_log_exp_and_others")
        if both is None:
            return tabs
        claimed = [
            mybir.ActivationFunctionType.Exp,
            mybir.ActivationFunctionType.Ln,
            mybir.ActivationFunctionType.Copy,
            mybir.ActivationFunctionType.Identity,
        ]
        for name, s in tabs.items():
            if name != "natural_log_exp_and_others":
                for fn in claimed:
                    s.discard(fn)
        return tabs

    bacc.get_activation_tables = patched
    bacc._act_tables_patched = True


def build_nc(sc_bufs=2, oT_bufs=2, lw_bufs=2, exp_bufs=4, xt_bufs=2,
             q_bufs=2, k_bufs=2, v_bufs=2, psb_bufs=2, asb_bufs=3,
             ps_bufs=3):
    _patch_act_tables()
    nc = bacc.Bacc("TRN2", target_bir_lowering=False, debug=False)

    xt = nc.dram_tensor("xt", [D, T], BF16, kind="ExternalInput").ap()
    wq = nc.dram_tensor("wq", [D, QC], BF16, kind="ExternalInput").ap()
    wk = nc.dram_tensor("wk", [D, DH], BF16, kind="ExternalInput").ap()
    wv = nc.dram_tensor("wv", [D, DH], BF16, kind="ExternalInput").ap()
    wo = nc.dram_tensor("wo", [QC, D], BF16, kind="ExternalInput").ap()
    cos2 = nc.dram_tensor("cos2", [DH, S], BF16, kind="ExternalInput").ap()
    sin2 = nc.dram_tensor("sin2", [DH, S], BF16, kind="ExternalInput").ap()
    m01 = nc.dram_tensor("m01", [4, 128, 512], BF16, kind="ExternalInput").ap()
    out = nc.dram_tensor("out", [T, D], F32, kind="ExternalOutput").ap()

    scale = 1.0 / math.sqrt(DH)
    Exp = mybir.ActivationFunctionType.Exp
    Ln = mybir.ActivationFunctionType.Ln

    with tile.TileContext(nc) as tc, ExitStack() as ctx:
        persist = ctx.enter_context(tc.tile_pool(name="persist", bufs=1))

        # --- resident weights / tables. Two HWDGE rings: xt slices (and
        # rope swaps / out) go on the sync(SP) ring; everything the first
        # si iteration doesn't immediately need rides the ACT ring so
        # xt slice 0 isn't queued behind 6MB of weights. ---
        wk_sb = persist.tile([128, KD, DH], BF16, tag="wk")
        nc.sync.dma_start(wk_sb[:], wk.rearrange("(o p) c -> p o c", p=128))
        cos_sb = persist.tile([128, S], BF16, tag="cos")
        nc.scalar.dma_start(cos_sb[:], cos2)
        sin_sb = persist.tile([128, S], BF16, tag="sin")
        nc.scalar.dma_start(sin_sb[:], sin2)
        wv_sb = persist.tile([128, KD, DH], BF16, tag="wv")
        nc.scalar.dma_start(wv_sb[:], wv.rearrange("(o p) c -> p o c", p=128))
        wq_sb = persist.tile([128, KD, QC], BF16, tag="wq")
        wq_r = wq.rearrange("(o p) c -> p o c", p=128)
        for h in range(HL):
            # per-head slices so q(h=0) of si=0 doesn't wait for all of wq
            nc.scalar.dma_start(
                wq_sb[:, :, bass.ts(h, DH)], wq_r[:, :, bass.ts(h, DH)]
            )
        msk_sb = persist.tile([128, 4, 512], BF16, tag="m01")
        nc.scalar.dma_start(msk_sb[:], m01.rearrange("r p q -> p r q"))
        wo_sb = persist.tile([128, HL, D], BF16, tag="wo")
        nc.scalar.dma_start(wo_sb[:], wo.rearrange("(o p) f -> p o f", p=128))
        ones_sb = persist.tile([128, 128], BF16, tag="ones")
        nc.vector.memset(ones_sb[:], 1.0)

        # --- resident activations (one tile per (h,b) / per b so phase C's
        # first reads only depend on the exact producer, not all of phase B) ---
        qT_sb = {
            (h, b): persist.tile([128, S], BF16, tag=f"qT{h}{b}",
                                 name=f"qT_sb{h}{b}")
            for h in range(HL) for b in range(BL)
        }
        kT_sb = {
            b: persist.tile([128, S], BF16, tag=f"kT{b}", name=f"kT_sb{b}")
            for b in range(BL)
        }
        v_sb = {
            b: persist.tile([128, NT128, DH], BF16, tag=f"v{b}", name=f"v_sb{b}")
            for b in range(BL)
        }

        # ---------------- phase B: projections + RoPE ----------------
        with tc.tile_pool(name="proj_sb", bufs=psb_bufs) as psb, \
             tc.tile_pool(name="proj_ps", bufs=2, space="PSUM") as pps:

            def rope(dst, raw_ps, pos_sl):
                """dst[128,512] <- RoPE(raw_ps[128,512] psum), via bf16 sbuf."""
                raw = psb.tile([128, 512], BF16, tag="rraw")
                nc.scalar.copy(raw[:], raw_ps[:])
                swp = psb.tile([128, 512], BF16, tag="rswp")
                nc.sync.dma_start(swp[0:64, :], raw[64:128, :])
                nc.sync.dma_start(swp[64:128, :], raw[0:64, :])
                t1 = psb.tile([128, 512], BF16, tag="rt1")
                nc.vector.tensor_mul(t1[:], raw[:], cos_sb[:, pos_sl])
                t2 = psb.tile([128, 512], BF16, tag="rt2")
                nc.vector.tensor_mul(t2[:], swp[:], sin_sb[:, pos_sl])
                nc.vector.tensor_add(dst, t1[:], t2[:])

            for si in range(T // 512):
                b, sl = divmod(si, NSL)
                pos_sl = bass.ts(sl, 512)
                xt_sl = psb.tile([128, KD, 512], BF16, tag="xt", bufs=xt_bufs)
                xt_r = xt[:, bass.ts(si, 512)].rearrange("(o p) t -> p o t", p=128)
                if si == 0:
                    # quarter chunks: the k matmul accumulation starts once
                    # the first 0.5MB lands instead of after the full 2MB
                    for o4 in range(4):
                        nc.sync.dma_start(
                            xt_sl[:, bass.ts(o4, 4), :], xt_r[:, bass.ts(o4, 4), :]
                        )
                else:
                    nc.sync.dma_start(xt_sl[:], xt_r)
                # k first: needs only wk (+cos/sin) resident
                k_ps = pps.tile([128, 512], F32, tag="k", bufs=k_bufs)
                for o in range(KD):
                    nc.tensor.matmul(
                        k_ps[:], wk_sb[:, o, :], xt_sl[:, o, :],
                        start=(o == 0), stop=(o == KD - 1),
                    )
                rope(kT_sb[b][:, pos_sl], k_ps, pos_sl)
                for jt in range(4):
                    v_ps = pps.tile([128, DH], F32, tag="v", bufs=v_bufs)
                    for o in range(KD):
                        nc.tensor.matmul(
                            v_ps[:], xt_sl[:, o, bass.ts(jt, 128)], wv_sb[:, o, :],
                            start=(o == 0), stop=(o == KD - 1),
                        )
                    nc.scalar.copy(v_sb[b][:, 4 * sl + jt, :], v_ps[:])
                for h in range(HL):
                    q_ps = pps.tile([128, 512], F32, tag="q", bufs=q_bufs)
                    for o in range(KD):
                        nc.tensor.matmul(
                            q_ps[:], wq_sb[:, o, bass.ts(h, DH)], xt_sl[:, o, :],
                            start=(o == 0), stop=(o == KD - 1),
                        )
                    rope(qT_sb[h, b][:, pos_sl], q_ps, pos_sl)

        # ---------------- phase C: attention + wo ----------------
        with tc.tile_pool(name="att_sb", bufs=asb_bufs) as asb, \
             tc.tile_pool(name="att_sb2", bufs=2) as asb2, \
             tc.tile_pool(name="att_ps", bufs=2, space="PSUM") as aps:

            def wo_group(b, qs, aoT, nt, od, tag="wo", copy_eng="v"):
                """one wo matmul group + out-DMA for 128 tokens x 512 dims."""
                w_ps = aps.tile([128, 512], F32, tag=tag, name="w_ps", bufs=1)
                for c in range(HL):
                    nc.tensor.matmul(
                        w_ps[:], aoT[:, c, bass.ts(nt, 128)],
                        wo_sb[:, c, bass.ts(od, 512)],
                        start=(c == 0), stop=(c == HL - 1),
                    )
                o_sb = asb.tile([128, 512], F32, tag="out")
                if copy_eng == "v":
                    nc.vector.tensor_copy(o_sb[:], w_ps[:])
                else:
                    nc.scalar.copy(o_sb[:], w_ps[:])
                nc.sync.dma_start(
                    out[
                        bass.ds(b * S + qs * 512 + nt * 128, 128),
                        bass.ts(od, 512),
                    ],
                    o_sb[:],
                )

            # wo groups of block i are spread evenly between the attention
            # pairs of block i+1 so PE always has independent matmul work
            # while the softmax pipeline (exp on ACT, adds/recip on DVE)
            # catches up; copies alternate DVE/ACT to balance those engines
            pending_wo = []
            for b in range(BL):
                for qs in range(NSL):
                    q_sl = bass.ts(qs, 512)
                    nk = 4 * qs + 4
                    npair = nk // 2
                    nquad = nk // 4
                    total_pairs = HL * npair
                    # emit pending_wo[i] when the block-wide pair counter
                    # reaches emit_at[i] (even distribution across the block)
                    emit_at = [
                        (i * total_pairs) // max(len(pending_wo), 1)
                        for i in range(len(pending_wo))
                    ]
                    pair_ctr = 0
                    aoT = asb2.tile([128, HL, 512], BF16, tag="aoT")
                    for h in range(HL):
                        oT_ps = aps.tile([128, 512], F32, tag="oT", bufs=oT_bufs)
                        l_ps = aps.tile(
                            [128, 512], F32, tag="l", name="l_ps", bufs=1
                        )
                        # ones-matmul count for this head: quads are further
                        # paired into octs so the row-sum matmuls shrink
                        n_lmm = (nquad + 1) // 2
                        lmm_i = 0
                        pend = None  # pair half-sum awaiting its quad partner
                        pend_q = None  # quad sum awaiting its oct partner
                        for p in range(npair):
                            j0 = 2 * p
                            s_ps = aps.tile([128, 1024], F32, tag="sc", bufs=sc_bufs)
                            e_sb = asb.tile(
                                [128, 1024], BF16, tag="exp", bufs=exp_bufs
                            )
                            for t in range(2):
                                j = j0 + t
                                nc.tensor.matmul(
                                    s_ps[:, bass.ts(t, 512)],
                                    kT_sb[b][:, bass.ts(j, 128)],
                                    qT_sb[h, b][:, q_sl],
                                    start=True, stop=True, skip_group_check=True,
                                )
                            # one exp for both k-tiles: [128,1024]
                            nc.scalar.activation(e_sb[:], s_ps[:], Exp, scale=scale)
                            for t in range(2):
                                j = j0 + t
                                r = j - 4 * qs
                                if r >= 0:
                                    # zero the causally-invalid region
                                    # (cols [0, 128(r+1)) of this half)
                                    w = 128 * (r + 1)
                                    sl_ = bass.ds(512 * t, w)
                                    nc.vector.tensor_mul(
                                        e_sb[:, sl_], e_sb[:, sl_],
                                        msk_sb[:, r, 0:w],
                                    )
                            for t in range(2):
                                j = j0 + t
                                r = j - 4 * qs
                                qlo = 128 * r if r > 0 else 0
                                nc.tensor.matmul(
                                    oT_ps[:, qlo:], v_sb[b][:, j, :],
                                    e_sb[:, bass.ds(512 * t + qlo, 512 - qlo)],
                                    start=(j == 0), stop=(j == nk - 1),
                                    skip_group_check=True,
                                )
                            # softmax denominator: pair/quad tree on DVE,
                            # one accumulated ones-matmul per quad
                            t1 = asb.tile([128, 512], BF16, tag="ps1", bufs=ps_bufs)
                            nc.vector.tensor_add(
                                t1[:], e_sb[:, 0:512], e_sb[:, 512:1024]
                            )
                            if pend is None:
                                pend = t1
                            else:
                                t3 = asb.tile([128, 512], BF16, tag="ps2", bufs=2)
                                nc.vector.tensor_add(t3[:], pend[:], t1[:])
                                pend = None
                                if pend_q is None and p == npair - 1:
                                    # odd quad count: matmul it directly
                                    nc.tensor.matmul(
                                        l_ps[:], ones_sb[:], t3[:],
                                        start=(lmm_i == 0),
                                        stop=(lmm_i == n_lmm - 1),
                                        skip_group_check=True,
                                    )
                                    lmm_i += 1
                                elif pend_q is None:
                                    pend_q = t3
                                else:
                                    t7 = asb.tile(
                                        [128, 512], BF16, tag="ps3", bufs=2
                                    )
                                    nc.vector.tensor_add(t7[:], pend_q[:], t3[:])
                                    pend_q = None
                                    nc.tensor.matmul(
                                        l_ps[:], ones_sb[:], t7[:],
                                        start=(lmm_i == 0),
                                        stop=(lmm_i == n_lmm - 1),
                                        skip_group_check=True,
                                    )
                                    lmm_i += 1
                            while pending_wo and emit_at and pair_ctr >= emit_at[0]:
                                emit_at.pop(0)
                                pending_wo.pop(0)()
                            pair_ctr += 1
                        rc_sb = asb.tile([128, 512], F32, tag="rc")
                        nc.vector.reciprocal_approx_fast(rc_sb[:], l_ps[:])
                        nc.vector.tensor_mul(aoT[:, h, :], oT_ps[:], rc_sb[:])
                    while pending_wo:
                        pending_wo.pop(0)()
                    pending_wo = [
                        (lambda b=b, qs=qs, aoT=aoT, nt=nt, od=od, i=4 * nt + od:
                         wo_group(b, qs, aoT, nt, od,
                                  copy_eng="v" if i % 2 else "s"))
                        for nt in range(4) for od in range(4)
                    ]
            # keep-warm matmuls: PE idles ~4-5µs here waiting for the last
            # head's softmax tail before the final wo flush; that idle
            # re-throttles the HAM clock gate to 1.2GHz for the whole flush.
            # These dummies (results never read) fill the idle window.
            warm_ps = aps.tile(
                [128, 512], F32, tag="sc", name="warm_ps", bufs=sc_bufs
            )
            for w in range(20):
                nc.tensor.matmul(
                    warm_ps[:], ones_sb[:], kT_sb[0][:, 0:512],
                    start=True, stop=True, skip_group_check=True,
                )
            # final block's wo: alternate the two single-buf PSUM tags so
            # group i+1's matmuls overlap group i's copy (no attention work
            # is left to fill the gap here)
            for i, (nt, od) in enumerate(
                (nt, od) for nt in range(4) for od in range(4)
            ):
                wo_group(BL - 1, NSL - 1, aoT, nt, od,
                         tag="wo" if i % 2 else "l",
                         copy_eng="v" if i % 2 else "s")
    nc.finalize()
    return nc


_NC_CACHE = {}


def _get_nc():
    if "nc" not in _NC_CACHE:
        _NC_CACHE["nc"] = build_nc()
    return _NC_CACHE["nc"]


def kernel(x, freqs_cos, freqs_sin, wq, wk, wv, wo):
    x = np.asarray(x)
    freqs_cos = np.asarray(freqs_cos)
    freqs_sin = np.asarray(freqs_sin)
    wq = np.asarray(wq)
    wk = np.asarray(wk)
    wv = np.asarray(wv)
    wo = np.asarray(wo)
    bf = ml_dtypes.bfloat16
    perm = np.concatenate([np.arange(0, DH, 2), np.arange(1, DH, 2)])

    wq_p = np.ascontiguousarray(
        wq.reshape(D, NH, DH)[:, :, perm].reshape(D, NH * DH)
    ).astype(bf)
    wk_p = np.ascontiguousarray(
        wk.reshape(D, NKV, DH)[:, :, perm].reshape(D, NKV * DH)
    ).astype(bf)
    wv_b = wv.astype(bf)
    wo_b = wo.astype(bf)

    cosT = freqs_cos.T  # [64, S]
    sinT = freqs_sin.T
    c2 = np.ascontiguousarray(np.concatenate([cosT, cosT], axis=0)).astype(bf)
    s2 = np.ascontiguousarray(np.concatenate([-sinT, sinT], axis=0)).astype(bf)

    # multiplicative mask[r][k, q] = 1 if causally valid (q - k - 128 r >= 0)
    # else 0; applied to e = exp(scores) on DVE
    kk = np.arange(128)[:, None]
    qq = np.arange(512)[None, :]
    masks = np.stack(
        [np.where(qq - kk - 128 * r >= 0, 1.0, 0.0) for r in range(4)]
    ).astype(bf)

    in_maps = []
    for core in range(8):
        dp, tp = divmod(core, TP)
        xs = x[dp * BL : (dp + 1) * BL].reshape(T, D)
        xt = np.ascontiguousarray(xs.T).astype(bf)
        in_maps.append(
            {
                "xt": xt,
                "wq": np.ascontiguousarray(wq_p[:, tp * QC : (tp + 1) * QC]),
                "wk": np.ascontiguousarray(wk_p[:, tp * DH : (tp + 1) * DH]),
                "wv": np.ascontiguousarray(wv_b[:, tp * DH : (tp + 1) * DH]),
                "wo": np.ascontiguousarray(wo_b[tp * QC : (tp + 1) * QC, :]),
                "cos2": c2,
                "sin2": s2,
                "m01": masks,
            }
        )

    nc = _get_nc()
    res = run_bass_kernel_spmd(nc, in_maps, core_ids=list(range(8)))
    _NC_CACHE["last_results"] = res

    full = np.zeros((B, S, D), dtype=np.float32)
    for core in range(8):
        dp = core // TP
        full[dp * BL : (dp + 1) * BL] += (
            res.results[core]["out"].astype(np.float32).reshape(BL, S, D)
        )
    return full


# revision 31
# speedup vs baseline: 1.0170x; 1.0013x over previous
"""GQA attention layer (B=4, S=2048, D=2048, 16 heads / 4 KV heads, RoPE,
causal) on 8 trn2 NeuronCores.

Sharding: TP=4 over KV-head groups x DP=2 over batch. Each core handles 2
batches and one KV group (4 q heads + 1 kv head), computes a partial
(head-group) contribution to out = attn @ wo; host sums the 4 partials per
batch group.

Device layout choices:
  - host pre-casts to bf16, pre-transposes x -> xT [D, T], and permutes
    wq/wk columns per head to "evens then odds" so RoPE becomes rotate-half.
  - q,k are produced transposed ([dh, tok]) straight from the projection
    matmuls; v is produced natural ([tok, dh]).
  - RoPE in transposed layout: rot = q * C2 + swap_halves(q) * S2 with
    C2 = [cos;cos], S2 = [-sin;+sin]; the half swap is 2 SBUF->SBUF DMAs.
  - attention: scoresT = kT_tile.T @ qT (k on partitions), full 512-wide
    score tiles paired two-per-PSUM-group so one ACT exp covers [128,1024]
    (halves the 352-cycle ACT fixed cost). Causal masking is a 0/1
    multiplicative mask applied to e on the Vector engine (keeps PE free).
    Softmax denominator: e tiles pair+quad-added on DVE (bf16), one
    accumulated ones-matmul per quad; 1/l via ACT ln -> exp(-x).
  - wo PSUM->SBUF copies on DVE; wo PSUM tiles share a 2-buf pool with the
    l tiles so wo groups double-buffer (8 PSUM banks exactly).
"""

import math
from contextlib import ExitStack

import ml_dtypes
import numpy as np

import concourse.bass as bass
import concourse.mybir as mybir
import concourse.tile as tile
from concourse import bacc
from concourse.bass_utils import run_bass_kernel_spmd

BF16 = mybir.dt.bfloat16
F32 = mybir.dt.float32

# Full-problem constants (hardcoded per harness contract)
B, S, D = 4, 2048, 2048
NH, NKV, DH = 16, 4, 128
TP, DP = 4, 2
BL = B // DP          # batches per core
T = BL * S            # tokens per core
HL = NH // TP         # q heads per core
QC = HL * DH          # q cols per core
NT128 = S // 128      # 128-token tiles per batch (16)
NSL = S // 512        # 512-token slices per batch (4)
KD = D // 128         # contraction tiles for the projections (16)


def _patch_act_tables():
    """Make natural_log_exp_and_others the only set claiming Exp/Ln/Copy/
    Identity (everything this kernel runs on ACT) so the act-table-load pass
    emits exactly one load instead of switching sets between the projection
    phase (copies) and the attention phase (exp)."""
    if getattr(bacc, "_act_tables_patched", False):
        return
    orig = bacc.get_activation_tables

    def patched(arch):
        tabs = orig(arch)
        both = tabs.get("natural_log_exp_and_others")
        if both is None:
            return tabs
        claimed = [
            mybir.ActivationFunctionType.Exp,
            mybir.ActivationFunctionType.Ln,
            mybir.ActivationFunctionType.Copy,
            mybir.ActivationFunctionType.Identity,
        ]
        for name, s in tabs.items():
            if name != "natural_log_exp_and_others":
                for fn in claimed:
                    s.discard(fn)
        return tabs

    bacc.get_activation_tables = patched
    bacc._act_tables_patched = True


def build_nc(sc_bufs=2, oT_bufs=2, lw_bufs=2, exp_bufs=4, xt_bufs=2,
             q_bufs=2, k_bufs=2, v_bufs=2, psb_bufs=2, asb_bufs=3,
             ps_bufs=3):
    _patch_act_tables()
    nc = bacc.Bacc("TRN2", target_bir_lowering=False, debug=False)

    xt = nc.dram_tensor("xt", [D, T], BF16, kind="ExternalInput").ap()
    wq = nc.dram_tensor("wq", [D, QC], BF16, kind="ExternalInput").ap()
    wk = nc.dram_tensor("wk", [D, DH], BF16, kind="ExternalInput").ap()
    wv = nc.dram_tensor("wv", [D, DH], BF16, kind="ExternalInput").ap()
    wo = nc.dram_tensor("wo", [QC, D], BF16, kind="ExternalInput").ap()
    cos2 = nc.dram_tensor("cos2", [DH, S], BF16, kind="ExternalInput").ap()
    sin2 = nc.dram_tensor("sin2", [DH, S], BF16, kind="ExternalInput").ap()
    m01 = nc.dram_tensor("m01", [4, 128, 512], BF16, kind="ExternalInput").ap()
    out = nc.dram_tensor("out", [T, D], F32, kind="ExternalOutput").ap()

    scale = 1.0 / math.sqrt(DH)
    Exp = mybir.ActivationFunctionType.Exp
    Ln = mybir.ActivationFunctionType.Ln

    with tile.TileContext(nc) as tc, ExitStack() as ctx:
        persist = ctx.enter_context(tc.tile_pool(name="persist", bufs=1))

        # --- resident weights / tables. Two HWDGE rings: xt slices (and
        # rope swaps / out) go on the sync(SP) ring; everything the first
        # si iteration doesn't immediately need rides the ACT ring so
        # xt slice 0 isn't queued behind 6MB of weights. ---
        wk_sb = persist.tile([128, KD, DH], BF16, tag="wk")
        nc.sync.dma_start(wk_sb[:], wk.rearrange("(o p) c -> p o c", p=128))
        cos_sb = persist.tile([128, S], BF16, tag="cos")
        nc.scalar.dma_start(cos_sb[:], cos2)
        sin_sb = persist.tile([128, S], BF16, tag="sin")
        nc.scalar.dma_start(sin_sb[:], sin2)
        wv_sb = persist.tile([128, KD, DH], BF16, tag="wv")
        nc.scalar.dma_start(wv_sb[:], wv.rearrange("(o p) c -> p o c", p=128))
        wq_sb = persist.tile([128, KD, QC], BF16, tag="wq")
        wq_r = wq.rearrange("(o p) c -> p o c", p=128)
        for h in range(HL):
            # per-head slices so q(h=0) of si=0 doesn't wait for all of wq
            nc.scalar.dma_start(
                wq_sb[:, :, bass.ts(h, DH)], wq_r[:, :, bass.ts(h, DH)]
            )
        # m01 is first read in phase C (~200us in): keep it behind wq
        msk_sb = persist.tile([128, 4, 512], BF16, tag="m01")
        nc.scalar.dma_start(msk_sb[:], m01.rearrange("r p q -> p r q"))
        wo_sb = persist.tile([128, HL, D], BF16, tag="wo")
        nc.scalar.dma_start(wo_sb[:], wo.rearrange("(o p) f -> p o f", p=128))
        ones_sb = persist.tile([128, 128], BF16, tag="ones")
        nc.vector.memset(ones_sb[:], 1.0)

        # --- resident activations (one tile per (h,b) / per b so phase C's
        # first reads only depend on the exact producer, not all of phase B) ---
        qT_sb = {
            (h, b): persist.tile([128, S], BF16, tag=f"qT{h}{b}",
                                 name=f"qT_sb{h}{b}")
            for h in range(HL) for b in range(BL)
        }
        kT_sb = {
            b: persist.tile([128, S], BF16, tag=f"kT{b}", name=f"kT_sb{b}")
            for b in range(BL)
        }
        v_sb = {
            b: persist.tile([128, NT128, DH], BF16, tag=f"v{b}", name=f"v_sb{b}")
            for b in range(BL)
        }

        # ---------------- phase B: projections + RoPE ----------------
        with tc.tile_pool(name="proj_sb", bufs=psb_bufs) as psb, \
             tc.tile_pool(name="proj_ps", bufs=2, space="PSUM") as pps:

            def rope(dst, raw_ps, pos_sl):
                """dst[128,512] <- RoPE(raw_ps[128,512] psum), via bf16 sbuf."""
                raw = psb.tile([128, 512], BF16, tag="rraw")
                nc.scalar.copy(raw[:], raw_ps[:])
                swp = psb.tile([128, 512], BF16, tag="rswp")
                nc.sync.dma_start(swp[0:64, :], raw[64:128, :])
                nc.sync.dma_start(swp[64:128, :], raw[0:64, :])
                t1 = psb.tile([128, 512], BF16, tag="rt1")
                nc.vector.tensor_mul(t1[:], raw[:], cos_sb[:, pos_sl])
                t2 = psb.tile([128, 512], BF16, tag="rt2")
                nc.vector.tensor_mul(t2[:], swp[:], sin_sb[:, pos_sl])
                nc.vector.tensor_add(dst, t1[:], t2[:])

            def emit_warm(n):
                """Dummy matmuls (never read) that keep the PE HAM clock
                gate at 2.4GHz through the startup DMA-ramp stalls."""
                warm = pps.tile([128, 128], F32, tag="warm", name="warm", bufs=1)
                for _ in range(n):
                    nc.tensor.matmul(
                        warm[:], ones_sb[:], ones_sb[:],
                        start=True, stop=True, skip_group_check=True,
                    )

            emit_warm(40)
            for si in range(T // 512):
                b, sl = divmod(si, NSL)
                pos_sl = bass.ts(sl, 512)
                xt_sl = psb.tile([128, KD, 512], BF16, tag="xt", bufs=xt_bufs)
                xt_r = xt[:, bass.ts(si, 512)].rearrange("(o p) t -> p o t", p=128)
                if si == 0:
                    # quarter chunks: the k matmul accumulation starts once
                    # the first 0.5MB lands instead of after the full 2MB
                    for o4 in range(4):
                        nc.sync.dma_start(
                            xt_sl[:, bass.ts(o4, 4), :], xt_r[:, bass.ts(o4, 4), :]
                        )
                else:
                    nc.sync.dma_start(xt_sl[:], xt_r)
                # k first: needs only wk (+cos/sin) resident
                k_ps = pps.tile([128, 512], F32, tag="k", bufs=k_bufs)
                for o in range(KD):
                    nc.tensor.matmul(
                        k_ps[:], wk_sb[:, o, :], xt_sl[:, o, :],
                        start=(o == 0), stop=(o == KD - 1),
                    )
                rope(kT_sb[b][:, pos_sl], k_ps, pos_sl)
                if si <= 1:
                    emit_warm(10)
                for jt in range(4):
                    v_ps = pps.tile([128, DH], F32, tag="v", bufs=v_bufs)
                    for o in range(KD):
                        nc.tensor.matmul(
                            v_ps[:], xt_sl[:, o, bass.ts(jt, 128)], wv_sb[:, o, :],
                            start=(o == 0), stop=(o == KD - 1),
                        )
                    nc.scalar.copy(v_sb[b][:, 4 * sl + jt, :], v_ps[:])
                if si <= 1:
                    emit_warm(10)
                for h in range(HL):
                    q_ps = pps.tile([128, 512], F32, tag="q", bufs=q_bufs)
                    for o in range(KD):
                        nc.tensor.matmul(
                            q_ps[:], wq_sb[:, o, bass.ts(h, DH)], xt_sl[:, o, :],
                            start=(o == 0), stop=(o == KD - 1),
                        )
                    rope(qT_sb[h, b][:, pos_sl], q_ps, pos_sl)

        # ---------------- phase C: attention + wo ----------------
        with tc.tile_pool(name="att_sb", bufs=asb_bufs) as asb, \
             tc.tile_pool(name="att_sb2", bufs=2) as asb2, \
             tc.tile_pool(name="att_ps", bufs=2, space="PSUM") as aps:

            def wo_group(b, qs, aoT, nt, od, tag="wo", copy_eng="v"):
                """one wo matmul group + out-DMA for 128 tokens x 512 dims."""
                w_ps = aps.tile([128, 512], F32, tag=tag, name="w_ps", bufs=1)
                for c in range(HL):
                    nc.tensor.matmul(
                        w_ps[:], aoT[:, c, bass.ts(nt, 128)],
                        wo_sb[:, c, bass.ts(od, 512)],
                        start=(c == 0), stop=(c == HL - 1),
                    )
                o_sb = asb.tile([128, 512], F32, tag="out")
                if copy_eng == "v":
                    nc.vector.tensor_copy(o_sb[:], w_ps[:])
                else:
                    nc.scalar.copy(o_sb[:], w_ps[:])
                nc.sync.dma_start(
                    out[
                        bass.ds(b * S + qs * 512 + nt * 128, 128),
                        bass.ts(od, 512),
                    ],
                    o_sb[:],
                )

            # wo groups of block i are spread evenly between the attention
            # pairs of block i+1 so PE always has independent matmul work
            # while the softmax pipeline (exp on ACT, adds/recip on DVE)
            # catches up; copies alternate DVE/ACT to balance those engines
            pending_wo = []
            for b in range(BL):
                for qs in range(NSL):
                    q_sl = bass.ts(qs, 512)
                    nk = 4 * qs + 4
                    npair = nk // 2
                    nquad = nk // 4
                    total_pairs = HL * npair
                    # emit pending_wo[i] when the block-wide pair counter
                    # reaches emit_at[i] (even distribution across the block)
                    emit_at = [
                        (i * total_pairs) // max(len(pending_wo), 1)
                        for i in range(len(pending_wo))
                    ]
                    pair_ctr = 0
                    aoT = asb2.tile([128, HL, 512], BF16, tag="aoT")
                    for h in range(HL):
                        oT_ps = aps.tile([128, 512], F32, tag="oT", bufs=oT_bufs)
                        l_ps = aps.tile(
                            [128, 512], F32, tag="l", name="l_ps", bufs=1
                        )
                        # ones-matmul count for this head: quads are further
                        # paired into octs so the row-sum matmuls shrink
                        n_lmm = (nquad + 1) // 2
                        lmm_i = 0
                        pend = None  # pair half-sum awaiting its quad partner
                        pend_q = None  # quad sum awaiting its oct partner
                        for p in range(npair):
                            j0 = 2 * p
                            s_ps = aps.tile([128, 1024], F32, tag="sc", bufs=sc_bufs)
                            e_sb = asb.tile(
                                [128, 1024], BF16, tag="exp", bufs=exp_bufs
                            )
                            for t in range(2):
                                j = j0 + t
                                nc.tensor.matmul(
                                    s_ps[:, bass.ts(t, 512)],
                                    kT_sb[b][:, bass.ts(j, 128)],
                                    qT_sb[h, b][:, q_sl],
                                    start=True, stop=True, skip_group_check=True,
                                )
                            # one exp for both k-tiles: [128,1024]
                            nc.scalar.activation(e_sb[:], s_ps[:], Exp, scale=scale)
                            for t in range(2):
                                j = j0 + t
                                r = j - 4 * qs
                                if r >= 0:
                                    # zero the causally-invalid region
                                    # (cols [0, 128(r+1)) of this half)
                                    w = 128 * (r + 1)
                                    sl_ = bass.ds(512 * t, w)
                                    nc.vector.tensor_mul(
                                        e_sb[:, sl_], e_sb[:, sl_],
                                        msk_sb[:, r, 0:w],
                                    )
                            for t in range(2):
                                j = j0 + t
                                r = j - 4 * qs
                                qlo = 128 * r if r > 0 else 0
                                nc.tensor.matmul(
                                    oT_ps[:, qlo:], v_sb[b][:, j, :],
                                    e_sb[:, bass.ds(512 * t + qlo, 512 - qlo)],
                                    start=(j == 0), stop=(j == nk - 1),
                                    skip_group_check=True,
                                )
                            # softmax denominator: pair/quad tree on DVE,
                            # one accumulated ones-matmul per quad
                            t1 = asb.tile([128, 512], BF16, tag="ps1", bufs=ps_bufs)
                            nc.vector.tensor_add(
                                t1[:], e_sb[:, 0:512], e_sb[:, 512:1024]
                            )
                            if pend is None:
                                pend = t1
                            else:
                                t3 = asb.tile([128, 512], BF16, tag="ps2", bufs=2)
                                nc.vector.tensor_add(t3[:], pend[:], t1[:])
                                pend = None
                                if pend_q is None and p == npair - 1:
                                    # odd quad count: matmul it directly
                                    nc.tensor.matmul(
                                        l_ps[:], ones_sb[:], t3[:],
                                        start=(lmm_i == 0),
                                        stop=(lmm_i == n_lmm - 1),
                                        skip_group_check=True,
                                    )
                                    lmm_i += 1
                                elif pend_q is None:
                                    pend_q = t3
                                else:
                                    t7 = asb.tile(
                                        [128, 512], BF16, tag="ps3", bufs=2
                                    )
                                    nc.vector.tensor_add(t7[:], pend_q[:], t3[:])
                                    pend_q = None
                                    nc.tensor.matmul(
                                        l_ps[:], ones_sb[:], t7[:],
                                        start=(lmm_i == 0),
                                        stop=(lmm_i == n_lmm - 1),
                                        skip_group_check=True,
                                    )
                                    lmm_i += 1
                            while pending_wo and emit_at and pair_ctr >= emit_at[0]:
                                emit_at.pop(0)
                                pending_wo.pop(0)()
                            pair_ctr += 1
                        rc_sb = asb.tile([128, 512], F32, tag="rc")
                        nc.vector.reciprocal_approx_fast(rc_sb[:], l_ps[:])
                        nc.vector.tensor_mul(aoT[:, h, :], oT_ps[:], rc_sb[:])
                    while pending_wo:
                        pending_wo.pop(0)()
                    pending_wo = [
                        (lambda b=b, qs=qs, aoT=aoT, nt=nt, od=od, i=4 * nt + od:
                         wo_group(b, qs, aoT, nt, od,
                                  copy_eng="v" if i % 2 else "s"))
                        for nt in range(4) for od in range(4)
                    ]
            # keep-warm matmuls: PE idles ~4-5µs here waiting for the last
            # head's softmax tail before the final wo flush; that idle
            # re-throttles the HAM clock gate to 1.2GHz for the whole flush.
            # These dummies (results never read) fill the idle window.
            warm_ps = aps.tile(
                [128, 512], F32, tag="sc", name="warm_ps", bufs=sc_bufs
            )
            for w in range(20):
                nc.tensor.matmul(
                    warm_ps[:], ones_sb[:], kT_sb[0][:, 0:512],
                    start=True, stop=True, skip_group_check=True,
                )
            # final block's wo: alternate the two single-buf PSUM tags so
            # group i+1's matmuls overlap group i's copy (no attention work
            # is left to fill the gap here)
            for i, (nt, od) in enumerate(
                (nt, od) for nt in range(4) for od in range(4)
            ):
                wo_group(BL - 1, NSL - 1, aoT, nt, od,
                         tag="wo" if i % 2 else "l",
                         copy_eng="v" if i % 2 else "s")
    nc.finalize()
    return nc


_NC_CACHE = {}


def _get_nc():
    if "nc" not in _NC_CACHE:
        _NC_CACHE["nc"] = build_nc()
    return _NC_CACHE["nc"]


def kernel(x, freqs_cos, freqs_sin, wq, wk, wv, wo):
    x = np.asarray(x)
    freqs_cos = np.asarray(freqs_cos)
    freqs_sin = np.asarray(freqs_sin)
    wq = np.asarray(wq)
    wk = np.asarray(wk)
    wv = np.asarray(wv)
    wo = np.asarray(wo)
    bf = ml_dtypes.bfloat16
    perm = np.concatenate([np.arange(0, DH, 2), np.arange(1, DH, 2)])

    wq_p = np.ascontiguousarray(
        wq.reshape(D, NH, DH)[:, :, perm].reshape(D, NH * DH)
    ).astype(bf)
    wk_p = np.ascontiguousarray(
        wk.reshape(D, NKV, DH)[:, :, perm].reshape(D, NKV * DH)
    ).astype(bf)
    wv_b = wv.astype(bf)
    wo_b = wo.astype(bf)

    cosT = freqs_cos.T  # [64, S]
    sinT = freqs_sin.T
    c2 = np.ascontiguousarray(np.concatenate([cosT, cosT], axis=0)).astype(bf)
    s2 = np.ascontiguousarray(np.concatenate([-sinT, sinT], axis=0)).astype(bf)

    # multiplicative mask[r][k, q] = 1 if causally valid (q - k - 128 r >= 0)
    # else 0; applied to e = exp(scores) on DVE
    kk = np.arange(128)[:, None]
    qq = np.arange(512)[None, :]
    masks = np.stack(
        [np.where(qq - kk - 128 * r >= 0, 1.0, 0.0) for r in range(4)]
    ).astype(bf)

    in_maps = []
    for core in range(8):
        dp, tp = divmod(core, TP)
        xs = x[dp * BL : (dp + 1) * BL].reshape(T, D)
        xt = np.ascontiguousarray(xs.T).astype(bf)
        in_maps.append(
            {
                "xt": xt,
                "wq": np.ascontiguousarray(wq_p[:, tp * QC : (tp + 1) * QC]),
                "wk": np.ascontiguousarray(wk_p[:, tp * DH : (tp + 1) * DH]),
                "wv": np.ascontiguousarray(wv_b[:, tp * DH : (tp + 1) * DH]),
                "wo": np.ascontiguousarray(wo_b[tp * QC : (tp + 1) * QC, :]),
                "cos2": c2,
                "sin2": s2,
                "m01": masks,
            }
        )

    nc = _get_nc()
    res = run_bass_kernel_spmd(nc, in_maps, core_ids=list(range(8)))
    _NC_CACHE["last_results"] = res

    full = np.zeros((B, S, D), dtype=np.float32)
    for core in range(8):
        dp = core // TP
        full[dp * BL : (dp + 1) * BL] += (
            res.results[core]["out"].astype(np.float32).reshape(BL, S, D)
        )
    return full
